# revision 17
# baseline (speedup 1.0000x reference)
"""Trainium2 Bass kernel for nn_MixBlock3D (MaxViT-style 3D mix block).

Reference pipeline:
  x = LN1(input)                                       [LN over C=256]
  xw = window_reverse(attn_w(window_partition(x)))     # 2x7x7 local windows
  y  = grid_reverse(attn_g(grid_partition(LN2(xw)))) + xw
  s  = input + y
  y1 = x1 + conv(leaky(conv(x2)))       [reversible conv block, 128ch 3x3x3]
  y2 = x2 + conv(leaky(conv(y1)))
  out = concat(y1, y2)

The axon tunnel (~60 MB/s up, ~48 MB/s down, strictly serialized) is the
bottleneck, so the design minimizes wire bytes and keeps the wire busy:

  - ONE compiled SPMD program processes ONE batch on all 8 cores; the two
    batches are dispatched as two pipelined launches, so batch-1 upload
    overlaps batch-0 exec and batch-0 download overlaps batch-1 exec.
  - input is uploaded int8 (per-batch per-channel scales, 2 chunks per
    batch so host quantization overlaps the wire);
  - conv weights ride int8 with per-(in,out)-channel bf16 scales; attn
    weights + bias tables ride the same uint8 blob, AllGathered on
    device (1/8 uploaded per core);
  - the device returns r = out - input as int8 (+ per-row f32 scale);
    the host adds the f32 input back, so input quantization error
    cancels in the residual path and the int8 output scale covers only
    r's range.

Per-launch stages (connected on-device with AllGather collectives):
  A: dequant int8 input -> bf16; LN1 + window attention; shard = H
     window-row blocks (rows [7c,7c+7)).
  AG1: gather xw + dequantized input -> full volume on every core.
  B: LN2 + grid attention + residuals; shard = H residue (rows == c mod 8).
  AG2: gather s = input + y -> full volume on every core.
  C: reversible conv block; shard = H 7-row blocks, 4-row halo recompute.
"""

import contextlib
import os
import sys
import time
from collections import deque

import numpy as np

for _p in ("/opt/trn_rl_repo", os.path.expanduser("~/.axon_site/_ro/trn_rl_repo")):
    if os.path.isdir(_p) and _p not in sys.path:
        sys.path.insert(0, _p)

os.environ.setdefault("NEURON_RT_RESET_CORES", "1")

import ml_dtypes

import concourse.bass as bass
import concourse.tile as tile
from concourse import bacc
from concourse import mybir
from concourse.alu_op_type import AluOpType
from concourse.masks import make_identity

F32 = mybir.dt.float32
BF16 = mybir.dt.bfloat16
I8 = mybir.dt.int8
U8 = mybir.dt.uint8
AX = mybir.AxisListType
AF = mybir.ActivationFunctionType
BF16_NP = ml_dtypes.bfloat16
DS = bass.DynSlice

# ---------------- problem constants (hardcoded per spec) ----------------
B, C, D, H, W = 2, 256, 8, 56, 56
NUM_HEADS = 4
HEAD_DIM = 64
SCALE = HEAD_DIM ** -0.5
N_CORES = 8
NTOK = 98          # tokens per window (2*7*7)
NWIN = 32          # windows per core per launch (one batch)
T = NWIN * NTOK    # tokens per core = 3136
TTILE = 392        # token tile for LN / qk / proj stages (= 4 windows)
NTT = T // TTILE   # 8
LN_EPS = 1e-5
HQ = 7             # output H rows per conv core (one batch, 8 cores)
HALO = 4
HIN = HQ + 2 * HALO  # 15 input rows per conv core
WPAD = W + 2       # 58
HPAD = HIN + 2     # 17
DPAD = D + 2       # 10

# weight blob packing: uint8 [128, GBYTES], AG-sharded (byte offsets).
GB_WQKV = 0          # 1536 bf16 (3072 B)
GB_WPROJ = 3072      # 512 bf16: rows 0:64 = heads 0,1; 64:128 = heads 2,3
GB_GQKV = 4096       # 1536 bf16
GB_GPROJ = 7168      # 512 bf16
GB_BTW = 8192        # 392 bf16 (rows 0:98) window-attn bias table
GB_BTG = 8976        # 392 bf16 grid-attn bias table
GB_CONV = 9760       # 4 x 3456 int8 conv weights
GB_CWS = GB_CONV + 4 * 3456   # 23584: 4 x 128 bf16 per-(in,out) conv scales
GBYTES = GB_CWS + 4 * 256     # 24608
WC = GBYTES // N_CORES        # 3076 bytes per core

# svec small-vector f32 columns (tiny replicated tensor, per launch)
SV_LN1W, SV_LN1B, SV_LN2W, SV_LN2B = 0, 2, 4, 6
SV_WPB, SV_GPB = 8, 10
SV_CB = {"f1": 12, "f2": 13, "g1": 14, "g2": 15}
SV_RS = 16           # 2 cols: input dequant scale for channel half k
NSV = 20

DH = D // 2          # 4: input chunk D-half
ODC = 2 * D * HQ * W          # 6272 int8 output data cols per launch
OQC = ODC + 4                 # + 4 bytes carrying the f32 scale


def _rel_index():
    d, h, w = 2, 7, 7
    coords = np.stack(
        np.meshgrid(np.arange(d), np.arange(h), np.arange(w), indexing="ij")
    ).reshape(3, -1)
    rel = (coords[:, :, None] - coords[:, None, :]).transpose(1, 2, 0).copy()
    rel[:, :, 0] += d - 1
    rel[:, :, 1] += h - 1
    rel[:, :, 2] += w - 1
    rel[:, :, 0] *= (2 * h - 1) * (2 * w - 1)
    rel[:, :, 1] *= 2 * w - 1
    return rel.sum(-1)  # (98, 98) int


RPI = _rel_index()


# ======================================================================
# Attention compute (32 windows of 98 tokens, C-major token layout)
# ======================================================================
def _attn_compute(tc, ctx, xin_t, out_t, w_qkv, w_proj, lnw_t, lnb_t, pb_t,
                  btab, residual):
    """LN + windowed attention over the 32 windows in xin_t (SBUF,
    (128, 2, T) bf16 token-major). Writes out_t (same shape); if residual,
    out += xin."""
    nc = tc.nc
    ts = bass.ts
    lnp = ctx.enter_context(tc.tile_pool(name="lnp", bufs=3))
    lnx = ctx.enter_context(tc.tile_pool(name="lnx", bufs=3))
    chk = ctx.enter_context(tc.tile_pool(name="chk", bufs=4))
    winp = ctx.enter_context(tc.tile_pool(name="winp", bufs=3))
    aux = ctx.enter_context(tc.tile_pool(name="aux", bufs=1))
    # PSUM: exactly 8 banks total.
    ps = ctx.enter_context(tc.tile_pool(name="ps", bufs=1, space="PSUM"))
    ps2 = ctx.enter_context(tc.tile_pool(name="ps2", bufs=2, space="PSUM"))

    ident = aux.tile([128, 128], BF16)
    make_identity(nc, ident)
    ones_col = aux.tile([128, 1], BF16)
    nc.vector.memset(ones_col[:], 1.0)
    ones_row = aux.tile([1, 128], BF16)
    nc.vector.memset(ones_row[:], 1.0)
    eps_t = aux.tile([1, 1], F32)
    nc.vector.memset(eps_t[:], LN_EPS)

    for ti in range(NTT):
        sl = ts(ti, TTILE)
        # =========== LayerNorm on this token tile ===========
        xc = xin_t[:, :, sl]
        xsq = lnx.tile([128, 2, TTILE], BF16, tag="xsq")
        nc.scalar.activation(xsq[:], xc[:], AF.Square)
        p_sum = ps.tile([1, TTILE], F32, tag="stat_a")
        p_sumsq = ps.tile([1, TTILE], F32, tag="stat_b")
        for k in range(2):
            nc.tensor.matmul(p_sum[:], ones_col[:], xc[:, k, :],
                             start=(k == 0), stop=(k == 1))
            nc.tensor.matmul(p_sumsq[:], ones_col[:], xsq[:, k, :],
                             start=(k == 0), stop=(k == 1))
        mean = lnp.tile([1, TTILE], F32, tag="mean")
        nc.vector.tensor_scalar_mul(mean[:], p_sum[:], 1.0 / C)
        msq = lnp.tile([1, TTILE], F32, tag="msq")
        nc.vector.tensor_tensor(msq[:], mean[:], mean[:], AluOpType.mult)
        rstd = lnp.tile([1, TTILE], F32, tag="rstd")
        nc.vector.scalar_tensor_tensor(rstd[:], p_sumsq[:], 1.0 / C,
                                       msq[:], AluOpType.mult,
                                       AluOpType.subtract)
        nc.scalar.activation(rstd[:], rstd[:], AF.Sqrt, bias=eps_t[:])
        nc.vector.reciprocal(rstd[:], rstd[:])
        mrstd = lnp.tile([1, TTILE], F32, tag="mrstd")
        nc.vector.tensor_tensor(mrstd[:], mean[:], rstd[:], AluOpType.mult)
        rb = lnp.tile([1, TTILE], BF16, tag="rb")
        nc.vector.tensor_copy(rb[:], rstd[:])
        mb = lnp.tile([1, TTILE], BF16, tag="mb")
        nc.vector.tensor_copy(mb[:], mrstd[:])
        b_rstd = ps.tile([128, TTILE], F32, tag="bc_a")
        nc.tensor.matmul(b_rstd[:], ones_row[:], rb[:], start=True,
                         stop=True)
        b_mrstd = ps.tile([128, TTILE], F32, tag="bc_b")
        nc.tensor.matmul(b_mrstd[:], ones_row[:], mb[:], start=True,
                         stop=True)
        xn = chk.tile([128, 2, TTILE], BF16, tag="xn")
        for k in range(2):
            t1 = lnp.tile([128, TTILE], F32, tag="t1")
            nc.vector.tensor_tensor(t1[:], xc[:, k, :], b_rstd[:],
                                    AluOpType.mult)
            nc.vector.tensor_tensor(t1[:], t1[:], b_mrstd[:],
                                    AluOpType.subtract)
            nc.vector.tensor_scalar(xn[:, k, :], t1[:],
                                    lnw_t[:, k:k + 1], lnb_t[:, k:k + 1],
                                    AluOpType.mult, AluOpType.add)

        # =========== q/k per head (base-0 only) ===========
        qa = chk.tile([64, 4, TTILE], BF16, tag="qa")
        kb = chk.tile([64, 4, TTILE], BF16, tag="kb")
        for h in range(4):
            p_q = ps2.tile([64, TTILE], F32, tag="mm")
            for k in range(2):
                nc.tensor.matmul(p_q[:], w_qkv[:, k, ts(h, 64)],
                                 xn[:, k, :], start=(k == 0), stop=(k == 1))
            (nc.scalar.copy if h % 2 == 0 else
             nc.vector.tensor_copy)(qa[:, h, :], p_q[:])
            p_k = ps2.tile([64, TTILE], F32, tag="mm")
            for k in range(2):
                nc.tensor.matmul(p_k[:], w_qkv[:, k, 256 + 64 * h:320 + 64 * h],
                                 xn[:, k, :], start=(k == 0), stop=(k == 1))
            (nc.vector.tensor_copy if h % 2 == 0 else
             nc.scalar.copy)(kb[:, h, :], p_k[:])

        # =========== 4 windows in this tile ===========
        at_c = chk.tile([64, 4, TTILE], BF16, tag="at")
        for wj in range(4):
            wsl = ts(wj, NTOK)
            # v = xn_w^T @ Wv  -> (98 tok, 256)
            p_v = ps.tile([128, 256], F32, tag="bc_b")
            for k in range(2):
                nc.tensor.matmul(p_v[:98, :], xn[:, k, wsl],
                                 w_qkv[:, k, 512:768],
                                 start=(k == 0), stop=(k == 1))
            v_sb = winp.tile([128, 256], BF16, tag="v_sb")
            nc.vector.tensor_copy(v_sb[:98, :], p_v[:98, :])
            # scores per head (K=64, both operands base 0)
            p_s = ps.tile([128, 392], F32, tag="bc_a")
            for h in range(4):
                nc.tensor.matmul(p_s[:98, ts(h, 98)],
                                 qa[:, h, wsl], kb[:, h, wsl],
                                 start=True, stop=True)
            sc_b = winp.tile([98, 392], BF16, tag="sc_b")
            nc.vector.tensor_tensor(sc_b[:], p_s[:98, :], btab[:],
                                    AluOpType.add)
            probs = winp.tile([98, 392], BF16, tag="probs")
            nc.scalar.activation(probs[:], sc_b[:], AF.Exp)
            den = winp.tile([98, 4], F32, tag="den")
            nc.vector.tensor_reduce(
                den[:, :, None],
                probs[:].rearrange("p (h n) -> p h n", h=4),
                AX.X, AluOpType.add)
            rden = winp.tile([98, 4], F32, tag="rden")
            nc.vector.reciprocal(rden[:], den[:])
            for h in range(4):
                nc.gpsimd.tensor_scalar_mul(probs[:, ts(h, 98)],
                                            probs[:, ts(h, 98)],
                                            rden[:, h:h + 1])
            # aT per head (PE transpose); 4 heads share one psum bank
            p_at = ps.tile([128, 392], BF16, tag="win_at")
            for h in range(4):
                nc.tensor.transpose(p_at[:98, ts(h, 98)],
                                    probs[:, ts(h, 98)], ident[:98, :98])
            at_sb = winp.tile([98, 392], BF16, tag="at_sb")
            nc.scalar.copy(at_sb[:], p_at[:98, :])
            # attnOut^T per head: (64 d, 98 q) at col h*98, base 0
            p_o = ps.tile([64, 392], F32, tag="win_o")
            for h in range(4):
                nc.tensor.matmul(p_o[:, ts(h, 98)],
                                 v_sb[:98, ts(h, 64)], at_sb[:, ts(h, 98)],
                                 start=True, stop=True)
            nc.scalar.copy(
                at_c[:, :, wsl],
                p_o[:].rearrange("p (h n) -> p h n", h=4))

        # =========== output projection (+ residual) ===========
        for mc in range(2):
            p_p = ps2.tile([128, TTILE], F32, tag="mm")
            for h in range(4):
                nc.tensor.matmul(p_p[:], w_proj[:, h, ts(mc, 128)],
                                 at_c[:, h, :],
                                 start=(h == 0), stop=(h == 3))
            nc.scalar.activation(out_t[:, mc, sl], p_p[:], AF.Identity,
                                 bias=pb_t[:, mc:mc + 1])
            if residual:
                nc.gpsimd.tensor_tensor(out_t[:, mc, sl], out_t[:, mc, sl],
                                        xin_t[:, mc, sl], AluOpType.add)


# ======================================================================
# Conv compute (reversible conv block, two leaky-conv chains, 3x3x3)
# ======================================================================
def _hblocks(h0, h1):
    """Split rows [h0, h1) into blocks of >=5 rows (N=W*rows >= 280 > 256)."""
    n = h1 - h0
    out = []
    while n > 0:
        b = 8 if n >= 8 else n
        if n - b in (1, 2, 3, 4) and b == 8:
            b = n - 5 if n - 5 <= 8 else 8
        out.append((h0, b))
        h0 += b
        n -= b
    return out


def _conv3d_stage(tc, psp, w_t, src_pad, h0, h1, emit):
    """Accumulate 27-tap conv over src_pad into psum tiles; call
    emit(psum_ap, d, hb, nrows) for each output tile."""
    nc = tc.nc
    for d in range(D):
        for (hb, nr) in _hblocks(h0, h1):
            pt = psp.tile([128, 8 * W], F32, tag="cv")
            outap = pt[:, : nr * W].rearrange("p (h w) -> p h w", h=nr)
            first = True
            for kd in range(3):
                for kh in range(3):
                    for kw in range(3):
                        ki = (kd * 3 + kh) * 3 + kw
                        rhs = src_pad[:, d + kd, hb + kh:hb + kh + nr,
                                      kw:kw + W]
                        nc.tensor.matmul(
                            outap, w_t[:, ki, :], rhs,
                            start=first, stop=(ki == 26))
                        first = False
            emit(pt[:, : nr * W].rearrange("p (h w) -> p h w", h=nr), d, hb, nr)


def _conv_compute(tc, ctx, sxi, gathW, sv_t, vm, inx, outq):
    """Reversible conv block on the per-core slab sxi (DRAM internal,
    (128, 2, D, HIN, W) bf16: [:,0]=x1, [:,1]=x2, rows [7q-4, 7q+11)
    zero-padded at volume edges). Conv weights arrive int8 with
    per-(in,out) bf16 scales. Ships r = out - input (inx subtracted) as
    int8 rows + f32 scales."""
    nc = tc.nc
    singles = ctx.enter_context(tc.tile_pool(name="csing", bufs=1))
    wpool = ctx.enter_context(tc.tile_pool(name="cwp", bufs=2))
    padA = ctx.enter_context(tc.tile_pool(name="cpadA", bufs=1))
    padB = ctx.enter_context(tc.tile_pool(name="cpadB", bufs=1))
    sc = ctx.enter_context(tc.tile_pool(name="cscr", bufs=3))
    psp = ctx.enter_context(tc.tile_pool(name="cps", bufs=4, space="PSUM"))

    b_t = {}
    for name in ("f1", "f2", "g1", "g2"):
        b_t[name] = singles.tile([128, 1], F32, tag=f"b_{name}",
                                 name=f"b_{name}")
        nc.vector.tensor_copy(b_t[name][:], sv_t[:, SV_CB[name]:SV_CB[name] + 1])

    def load_w(idx):
        wq = wpool.tile([128, 27, 128], I8, tag="wq")
        _load_blob(nc, gathW, wq[:].rearrange("p a b -> p (a b)"),
                   GB_CONV + 3456 * idx, GB_CONV + 3456 * (idx + 1), dt=I8)
        s_t = wpool.tile([128, 128], BF16, tag="ws")
        _load_blob(nc, gathW, s_t[:], GB_CWS + 256 * idx,
                   GB_CWS + 256 * (idx + 1))
        wt = wpool.tile([128, 27, 128], BF16, tag="w")
        nc.vector.tensor_copy(wt[:].rearrange("p a b -> p (a b)"),
                              wq[:].rearrange("p a b -> p (a b)"))
        # dequant: per-(in partition, out column) scale, broadcast over taps
        nc.vector.tensor_tensor(
            wt[:], wt[:],
            s_t[:, None, :].to_broadcast((128, 27, 128)),
            AluOpType.mult)
        return wt

    def new_pad(pool, tag):
        t = pool.tile([128, DPAD, HPAD, WPAD], BF16, tag=tag)
        nc.vector.memset(t[:], 0.0)
        return t

    out_st = singles.tile([128, 2, D, HQ, W], BF16, tag="out_st")

    # ---- x2pad <- x2 slab ----
    x2pad = new_pad(padA, "pA")
    for d in range(D):
        nc.sync.dma_start(x2pad[:, 1 + d, 1:1 + HIN, 1:1 + W],
                          sxi[:, 1, d])

    # ---- f1 = leaky(conv(x2)+b) on rows [1,14) ----
    w_f1 = load_w(0)
    f1pad = new_pad(padB, "pB")

    def emit_leaky(bias, dstpad):
        def emit(pap, d, hb, nr):
            t = sc.tile([128, 8, W], BF16, tag="lk")
            tt = t[:, :nr, :]
            # 0.99*relu(z) with z = conv+b ; relu(0.99 z) == 0.99 relu(z)
            nc.scalar.activation(tt, pap, AF.Relu, bias=bias[:], scale=0.99)
            dst = dstpad[:, d + 1, hb + 1:hb + 1 + nr, 1:1 + W]
            # dst = 0.01*(conv) + relu_part ; then += 0.01*b
            nc.vector.scalar_tensor_tensor(dst, pap, 0.01, tt,
                                           AluOpType.mult, AluOpType.add)
            if hb < HALO or hb + nr > HALO + HQ:
                # zero out-of-volume rows (reference SAME-pad semantics)
                nc.vector.tensor_tensor(
                    dst, dst,
                    vm[:, hb + 1:hb + 1 + nr, None].to_broadcast(
                        (128, nr, W)), AluOpType.mult)
        return emit

    bias99_f1 = singles.tile([128, 1], F32, tag="b99f1")
    nc.vector.tensor_scalar_mul(bias99_f1[:], b_t["f1"][:], 0.99)
    _conv3d_stage(tc, psp, w_f1, x2pad, 1, HIN - 1, emit_leaky(bias99_f1, f1pad))

    # ---- y1 = x1 + conv(f1)+b on rows [2,13) ----
    w_f2 = load_w(1)
    y1pad = new_pad(padA, "pA")   # reuses x2pad slot after f1 done
    for d in range(D):
        nc.sync.dma_start(y1pad[:, 1 + d, 1:1 + HIN, 1:1 + W],
                          sxi[:, 0, d])

    def emit_y1(pap, d, hb, nr):
        dst = y1pad[:, d + 1, hb + 1:hb + 1 + nr, 1:1 + W]
        t = sc.tile([128, 8, W], BF16, tag="y1t")
        tt = t[:, :nr, :]
        nc.scalar.activation(tt, pap, AF.Identity, bias=b_t["f2"][:])
        nc.vector.tensor_tensor(dst, dst, tt, AluOpType.add)
        if hb < HALO or hb + nr > HALO + HQ:
            nc.vector.tensor_tensor(
                dst, dst,
                vm[:, hb + 1:hb + 1 + nr, None].to_broadcast((128, nr, W)),
                AluOpType.mult)

    _conv3d_stage(tc, psp, w_f2, f1pad, 2, HIN - 2, emit_y1)
    # stage y1 output rows [4,11)
    for d in range(D):
        nc.scalar.copy(out_st[:, 0, d], y1pad[:, 1 + d, 5:5 + HQ, 1:1 + W])

    # ---- g1 = leaky(conv(y1)+b) on rows [3,12) ----
    w_g1 = load_w(2)
    g1pad = new_pad(padB, "pB")
    bias99_g1 = singles.tile([128, 1], F32, tag="b99g1")
    nc.vector.tensor_scalar_mul(bias99_g1[:], b_t["g1"][:], 0.99)
    _conv3d_stage(tc, psp, w_g1, y1pad, 3, HIN - 3, emit_leaky(bias99_g1, g1pad))

    # ---- y2 = x2 + conv(g1)+b on rows [4,11) ----
    w_g2 = load_w(3)

    def emit_y2(pap, d, hb, nr):
        x2c = sc.tile([128, 8, W], BF16, tag="x2c")
        nc.sync.dma_start(x2c[:, :nr, :], sxi[:, 1, d, hb:hb + nr, :])
        t = sc.tile([128, 8, W], BF16, tag="y2t")
        tt = t[:, :nr, :]
        nc.scalar.activation(tt, pap, AF.Identity, bias=b_t["g2"][:])
        nc.vector.tensor_tensor(out_st[:, 1, d, hb - 4:hb - 4 + nr, :],
                                tt, x2c[:, :nr, :], AluOpType.add)

    _conv3d_stage(tc, psp, w_g2, g1pad, 4, HIN - 4, emit_y2)

    # ---- r = out - input (residual shipped; host adds f32 input) ----
    flat = out_st[:].rearrange("p a b c d -> p (a b c d)")
    nc.vector.tensor_tensor(flat, flat,
                            inx[:].rearrange("p a b c d -> p (a b c d)"),
                            AluOpType.subtract)

    # ---- int8 quantization: per-channel scale = absmax/127 ----
    oabs = singles.tile([128, 2 * D * HQ * W], BF16, tag="oabs")
    nc.scalar.activation(oabs[:], flat, AF.Abs)
    absm = singles.tile([128, 1], F32, tag="absm")
    nc.vector.tensor_reduce(absm[:], oabs[:], AX.X, AluOpType.max)
    nc.vector.tensor_scalar_add(absm[:], absm[:], 1e-20)
    osc = singles.tile([128, 1], F32, tag="osc")
    nc.vector.tensor_scalar_mul(osc[:], absm[:], 1.0 / 127.0)
    nc.sync.dma_start(outq[:, ODC:OQC].bitcast(F32), osc[:])
    rsc = singles.tile([128, 1], F32, tag="rsc")
    nc.vector.reciprocal(rsc[:], osc[:])
    q = singles.tile([128, ODC], I8, tag="q")
    nc.vector.tensor_scalar_mul(q[:], flat, rsc[:])
    nc.sync.dma_start(outq[:, 0:ODC], q[:])


# ======================================================================
# Merged program (one batch per launch)
# ======================================================================
def _load_blob(nc, gathW, dst_flat, b0, b1, prow=0, nrows=128, dt=BF16):
    """DMA global blob bytes [b0, b1) (may span AG rank chunks) into the
    flat SBUF destination (dtype dt). prow/nrows select blob partition
    rows. All offsets must be 4-byte aligned (WC % 4 == 0 guarantees
    chunk-boundary alignment)."""
    sz = {BF16: 2, F32: 4, I8: 1, U8: 1}[dt]
    off = 0
    g = b0
    while g < b1:
        r, lo = divmod(g, WC)
        take = min(WC - lo, b1 - g)
        src = gathW[r, prow:prow + nrows, lo:lo + take]
        if dt is not U8:
            src = src.bitcast(dt)
        nc.sync.dma_start(dst_flat[:, off:off + take // sz], src)
        off += take // sz
        g += take


def _vtab_np():
    """(128, 8, HPAD) bf16: vtab[:, q, lp] = 1 if padded-local row lp of
    conv-core q maps to a valid global H row."""
    v = np.zeros((8, HPAD), np.float32)
    for q in range(8):
        lo = HQ * q - HALO
        for lp in range(1, 1 + HIN):
            g = lo + (lp - 1)
            v[q, lp] = 1.0 if 0 <= g < H else 0.0
    return np.broadcast_to(v, (128, 8, HPAD)).astype(BF16_NP).copy()


def build_mix_program():
    nc = bacc.Bacc("TRN2", debug=False, enable_asserts=False, num_devices=8)
    xq = {}
    for dh in range(2):
        xq[dh] = nc.dram_tensor(f"xq{dh}", [128, 2, DH, 7, W], I8,
                                kind="ExternalInput").ap()
    wshb = nc.dram_tensor("wshb", [128, WC], U8, kind="ExternalInput").ap()
    svec = nc.dram_tensor("svec", [128, NSV], F32,
                          kind="ExternalInput").ap()
    outq = nc.dram_tensor("outq", [128, OQC], I8,
                          kind="ExternalOutput").ap()
    with tile.TileContext(nc) as tc:
        _mix_body(tc, xq, wshb, svec, outq)
    nc.compile()
    return nc


def _mix_body(tc, xq, wsh, svec, outq):
    nc = tc.nc
    ts = bass.ts
    rg = [list(range(N_CORES))]
    with contextlib.ExitStack() as ctx:
        dram = ctx.enter_context(tc.tile_pool(name="dram", bufs=1,
                                              space="DRAM"))
        glob = ctx.enter_context(tc.tile_pool(name="glob", bufs=1))

        bncW = dram.tile([128, WC], U8, tag="bncW")
        gathW = dram.tile([N_CORES, 128, WC], U8, tag="gathW",
                          addr_space="Shared")
        bncA = dram.tile([2, 128, 2, D, 7, W], BF16, tag="bncA")
        gathA = dram.tile([N_CORES, 2, 128, 2, D, 7, W], BF16,
                          tag="gathA", addr_space="Shared")
        xpad = dram.tile([2, 128, 2, D, H, W], BF16, tag="xpad")
        bncS = dram.tile([128, 2, D, 7, W], BF16, tag="bncS")
        gathS = dram.tile([N_CORES, 128, 2, D, 7, W], BF16,
                          tag="gathS", addr_space="Shared")
        spad = dram.tile([128, 2, D, H + 2 * HALO, W], BF16, tag="spad")
        sxi = dram.tile([128, 2, D, HIN, W], BF16, tag="sxi")
        vtabd = dram.tile([128, 8, HPAD], BF16, tag="vtabd")

        # constant table for the conv-edge mask (per-core row validity)
        vtab_h = nc.inline_tensor(_vtab_np(), name="vtab_const")
        nc.sync.dma_start(vtabd[:], vtab_h.ap())

        # small vectors (replicated f32)
        sv_t = glob.tile([128, NSV], F32)
        nc.sync.dma_start(sv_t[:], svec)

        # ---- weight AllGather (first: stage A needs wqkv) ----
        nc.sync.dma_start(bncW[:], wsh)
        nc.gpsimd.collective_compute(
            "AllGather", AluOpType.bypass, replica_groups=rg,
            ins=[bncW[:]], outs=[gathW[:]])

        # partition-id registers (gpsimd issues all dynamic DMAs)
        pid = nc.gpsimd.partition_id()

        # ================= stage A: LN1 + window attention =================
        with contextlib.ExitStack() as sctx:
            sa = sctx.enter_context(tc.tile_pool(name="sa", bufs=1))
            dq = sctx.enter_context(tc.tile_pool(name="dq", bufs=2))

            # ---- dequantize int8 input chunks into bncA[1] (bf16) ----
            for dh in range(2):
                xq_t = dq.tile([128, 2, DH * 7 * W], I8, tag="xq")
                nc.sync.dma_start(
                    xq_t[:],
                    xq[dh].rearrange("p k d h w -> p k (d h w)"))
                for k in range(2):
                    xdq = dq.tile([128, DH * 7 * W], BF16, tag="xdq")
                    nc.vector.tensor_scalar_mul(
                        xdq[:], xq_t[:, k, :],
                        sv_t[:, SV_RS + k:SV_RS + k + 1])
                    nc.sync.dma_start(
                        bncA[1, :, k, DH * dh:DH * dh + DH, :, :],
                        xdq[:].rearrange("p (d h w) -> p d h w",
                                         d=DH, h=7))

            w_qkv = sa.tile([128, 2, 768], BF16)
            _load_blob(nc, gathW, w_qkv[:].rearrange("p a b -> p (a b)"),
                       GB_WQKV, GB_WQKV + 3072)
            w_proj = sa.tile([64, 4, 256], BF16)
            wp_flat = w_proj[:].rearrange("p a b -> p (a b)")
            _load_blob(nc, gathW, wp_flat[:, 0:512],
                       GB_WPROJ, GB_WPROJ + 1024, prow=0, nrows=64)
            _load_blob(nc, gathW, wp_flat[:, 512:1024],
                       GB_WPROJ, GB_WPROJ + 1024, prow=64, nrows=64)
            btb16 = sa.tile([98, 392], BF16)
            _load_blob(nc, gathW, btb16[:], GB_BTW, GB_BTW + 784,
                       prow=0, nrows=98)
            btab = sa.tile([98, 392], F32)
            nc.vector.tensor_copy(btab[:], btb16[:])

            xin_t = sa.tile([128, 2, T], BF16)
            for db in range(4):
                for wb in range(8):
                    w = db * 8 + wb
                    for k in range(2):
                        nc.sync.dma_start(
                            xin_t[:, k, ts(w, NTOK)].rearrange(
                                "p (dd hh ww) -> p dd hh ww", dd=2, hh=7),
                            bncA[1, :, k, 2 * db:2 * db + 2, :,
                                 7 * wb:7 * wb + 7])
            out_t = sa.tile([128, 2, T], BF16)
            _attn_compute(tc, sctx, xin_t, out_t, w_qkv, w_proj,
                          sv_t[:, SV_LN1W:SV_LN1W + 2],
                          sv_t[:, SV_LN1B:SV_LN1B + 2],
                          sv_t[:, SV_WPB:SV_WPB + 2],
                          btab, residual=False)
            # scatter xw tokens into bncA[0] (raw row-major layout)
            for db in range(4):
                for wb in range(8):
                    w = db * 8 + wb
                    for k in range(2):
                        nc.sync.dma_start(
                            bncA[0, :, k, 2 * db:2 * db + 2, :,
                                 7 * wb:7 * wb + 7],
                            out_t[:, k, ts(w, NTOK)].rearrange(
                                "p (dd hh ww) -> p dd hh ww", dd=2, hh=7))

        # ---- AllGather stage-A output + dequantized input ----
        nc.gpsimd.collective_compute(
            "AllGather", AluOpType.bypass, replica_groups=rg,
            ins=[bncA[:]], outs=[gathA[:]])

        # ---- xpad: reassemble full volume in plain row-major H ----
        for src in range(2):
            for r in range(N_CORES):
                for k in range(2):
                    nc.sync.dma_start(
                        xpad[src, :, k, :, 7 * r:7 * r + 7, :],
                        gathA[r, src, :, k])

        # ================= stage B: LN2 + grid attention =================
        with contextlib.ExitStack() as sctx:
            sb = sctx.enter_context(tc.tile_pool(name="sb", bufs=1))
            g_qkv = sb.tile([128, 2, 768], BF16)
            _load_blob(nc, gathW, g_qkv[:].rearrange("p a b -> p (a b)"),
                       GB_GQKV, GB_GQKV + 3072)
            g_proj = sb.tile([64, 4, 256], BF16)
            gp_flat = g_proj[:].rearrange("p a b -> p (a b)")
            _load_blob(nc, gathW, gp_flat[:, 0:512],
                       GB_GPROJ, GB_GPROJ + 1024, prow=0, nrows=64)
            _load_blob(nc, gathW, gp_flat[:, 512:1024],
                       GB_GPROJ, GB_GPROJ + 1024, prow=64, nrows=64)
            gbt16 = sb.tile([98, 392], BF16)
            _load_blob(nc, gathW, gbt16[:], GB_BTG, GB_BTG + 784,
                       prow=0, nrows=98)
            gbtab = sb.tile([98, 392], F32)
            nc.vector.tensor_copy(gbtab[:], gbt16[:])

            # dynamic row-slab loads: rows pid, pid+8, ..., pid+48.
            # W padded to 57 so (7, 56) doesn't collapse to one dim --
            # symbolic DMAs need exactly matching src/dst shapes.
            xw_s = sb.tile([128, 2, D, 7, W + 1], BF16)
            in_s = sb.tile([128, 2, D, 7, W + 1], BF16)
            nc.vector.memset(xw_s[:], 0.0)
            nc.vector.memset(in_s[:], 0.0)
            for k in range(2):
                for d in range(D):
                    nc.gpsimd.dma_start(
                        xw_s[:, k, d, :, 0:W],
                        xpad[0, :, k, d, DS(pid, 7, 8), :])
                    nc.gpsimd.dma_start(
                        in_s[:, k, d, :, 0:W],
                        xpad[1, :, k, d, DS(pid, 7, 8), :])
            # token assembly (grid windows) via engine copies --
            # SBUF->SBUF DMA can't rebalance two symbolic APs
            xw_g = sb.tile([128, 2, T], BF16)
            for dd in range(4):      # i_Dd
                for ww in range(8):  # i_Ww
                    w = dd * 8 + ww
                    for k in range(2):
                        eng = nc.scalar if (w + k) % 2 else nc.vector
                        (eng.copy if eng is nc.scalar
                         else eng.tensor_copy)(
                            xw_g[:, k, ts(w, NTOK)].rearrange(
                                "p (a h c) -> p a h c", a=2, h=7),
                            xw_s[:, k, dd:dd + 5:4, :,
                                 ww:ww + 49:8])
            out_t = sb.tile([128, 2, T], BF16)
            _attn_compute(tc, sctx, xw_g, out_t, g_qkv, g_proj,
                          sv_t[:, SV_LN2W:SV_LN2W + 2],
                          sv_t[:, SV_LN2B:SV_LN2B + 2],
                          sv_t[:, SV_GPB:SV_GPB + 2],
                          gbtab, residual=True)
            # s = input + y: scatter y tokens back into the (reused) xw
            # slab, then add the raw-input rows
            for dd in range(4):
                for ww in range(8):
                    w = dd * 8 + ww
                    for k in range(2):
                        eng = nc.scalar if (w + k) % 2 else nc.vector
                        (eng.copy if eng is nc.scalar
                         else eng.tensor_copy)(
                            xw_s[:, k, dd:dd + 5:4, :,
                                 ww:ww + 49:8],
                            out_t[:, k, ts(w, NTOK)].rearrange(
                                "p (a h c) -> p a h c", a=2, h=7))
            nc.vector.tensor_tensor(
                xw_s[:].rearrange("p a b c d -> p (a b c d)"),
                xw_s[:].rearrange("p a b c d -> p (a b c d)"),
                in_s[:].rearrange("p a b c d -> p (a b c d)"),
                AluOpType.add)
            for k in range(2):
                for d in range(D):
                    nc.sync.dma_start(bncS[:, k, d],
                                      xw_s[:, k, d, :, 0:W])

        # ---- AllGather s = input + y ----
        nc.gpsimd.collective_compute(
            "AllGather", AluOpType.bypass, replica_groups=rg,
            ins=[bncS[:]], outs=[gathS[:]])

        # ---- spad: full s volume, H padded by HALO zeros both sides ----
        with tc.tile_pool(name="zp", bufs=1) as zp:
            zt = zp.tile([128, D, HALO, W], BF16)
            nc.vector.memset(zt[:], 0.0)
            for k in range(2):
                nc.sync.dma_start(spad[:, k, :, 0:HALO, :], zt[:])
                nc.sync.dma_start(
                    spad[:, k, :, HALO + H:2 * HALO + H, :], zt[:])
        for r in range(N_CORES):
            for k in range(2):
                for d in range(D):
                    nc.sync.dma_start(
                        spad[:, k, d, HALO + r:HALO + r + 49:8, :],
                        gathS[r, :, k, d])

        # ================= stage C: reversible conv block =================
        with contextlib.ExitStack() as sctx:
            q7 = nc.gpsimd.compute_val(pid * HQ)
            for k in range(2):
                for d in range(D):
                    nc.gpsimd.dma_start(
                        sxi[:, k, d],
                        spad[:, k, d, DS(q7, HIN), :])
            cvp = sctx.enter_context(tc.tile_pool(name="cvp", bufs=1))
            vm = cvp.tile([128, HPAD], BF16)
            nc.gpsimd.dma_start(
                vm[:], vtabd[:, DS(pid, 1), :].rearrange("p q l -> p (q l)"))
            # this core's own (dequantized) input rows, for r = out - input
            inx = cvp.tile([128, 2, D, HQ, W], BF16)
            for k in range(2):
                for d in range(D):
                    nc.gpsimd.dma_start(
                        inx[:, k, d],
                        xpad[1, :, k, d, DS(q7, HQ), :])
            _conv_compute(tc, sctx, sxi, gathW, sv_t, vm, inx, outq)


# ======================================================================
# Host side: packing, persistent PJRT launcher, kernel()
# ======================================================================
LAST_EXEC_NS = []
_STATE = {}


def _pack_blob(inputs):
    """(N_CORES*128, WC) uint8 AG-sharded weight blob."""
    blob = np.zeros((128, GBYTES), np.uint8)

    def put_bf16(off, arr):
        a = np.ascontiguousarray(arr.astype(BF16_NP))
        blob[:a.shape[0], off:off + 2 * a.shape[1]] = a.view(np.uint8)

    def qkv_block(wq_in):
        wq = wq_in.astype(np.float32).copy()
        wq[:256] *= SCALE
        return wq.T.reshape(2, 128, 768).transpose(1, 0, 2).reshape(128, 1536)

    def proj_block(wp):
        w4 = wp.astype(np.float32).T.reshape(4, 64, 256)
        top = np.concatenate([w4[0], w4[1]], axis=1)
        bot = np.concatenate([w4[2], w4[3]], axis=1)
        return np.concatenate([top, bot], axis=0)  # (128, 512)

    def btab_of(tbl):
        bt = np.asarray(tbl).astype(np.float32)[RPI]       # (98, 98, 4)
        return np.ascontiguousarray(
            bt.transpose(0, 2, 1).reshape(98, 392))

    put_bf16(GB_WQKV, qkv_block(inputs["wqkv"]))
    put_bf16(GB_WPROJ, proj_block(inputs["wprojw"]))
    put_bf16(GB_GQKV, qkv_block(inputs["gqkv"]))
    put_bf16(GB_GPROJ, proj_block(inputs["gprojw"]))
    put_bf16(GB_BTW, btab_of(inputs["wbias"]))
    put_bf16(GB_BTG, btab_of(inputs["gbias"]))

    for i, wk in enumerate(("f1c1w", "f1c2w", "g1c1w", "g1c2w")):
        # (in, kd, kh, kw, out); scale per (in, out) group of 27 taps
        wt = inputs[wk].astype(np.float32).transpose(1, 2, 3, 4, 0)
        am = np.maximum(np.abs(wt).max(axis=(1, 2, 3)), 1e-30)  # (in, out)
        q = np.rint(wt * (127.0 / am)[:, None, None, None, :])
        blob[:, GB_CONV + 3456 * i:GB_CONV + 3456 * (i + 1)] = \
            q.astype(np.int8).reshape(128, 3456).view(np.uint8)
        put_bf16(GB_CWS + 256 * i, am / 127.0)

    wsh = np.ascontiguousarray(
        blob.reshape(128, N_CORES, WC).transpose(1, 0, 2)
    ).reshape(N_CORES * 128, WC)
    return wsh


def _make_svec(inputs, rs):
    """(N_CORES*128, NSV) f32 replicated small-vector tensor (per batch)."""
    sv = np.zeros((128, NSV), np.float32)

    def put2(col, vec):
        sv[:, col:col + 2] = vec.astype(np.float32).reshape(2, 128).T

    put2(SV_LN1W, inputs["n1w"]); put2(SV_LN1B, inputs["n1b"])
    put2(SV_LN2W, inputs["n2w"]); put2(SV_LN2B, inputs["n2b"])
    put2(SV_WPB, inputs["wprojb"]); put2(SV_GPB, inputs["gprojb"])
    for name, bk in (("f1", "f1c1b"), ("f2", "f1c2b"),
                     ("g1", "g1c1b"), ("g2", "g1c2b")):
        sv[:, SV_CB[name]] = inputs[bk].astype(np.float32)
    for k in range(2):
        sv[:, SV_RS + k] = rs[k]
    return np.tile(sv, (N_CORES, 1))


def _quant_batch(inp_b, tmp, u8buf):
    """Quantize one batch (256, D, H, W) f32 -> uint8 (round-to-nearest,
    +128 offset) with per-channel scale. Returns rs dict: {k: (128,)}
    dequant scales for channel half k."""
    mx = inp_b.max(axis=(1, 2, 3))
    mn = inp_b.min(axis=(1, 2, 3))
    am = np.maximum(np.maximum(mx, -mn), 1e-30)
    sc = 126.5 / am
    np.multiply(inp_b, sc[:, None, None, None], out=tmp)
    np.add(tmp, 128.5, out=tmp)
    np.copyto(u8buf, tmp, casting="unsafe")
    rs = 1.0 / sc
    return {0: rs[:128], 1: rs[128:]}


def _scatter_chunk(u8buf, dh, out_i8):
    """u8buf (256, D, H, W) -> chunk (N_CORES*128, 2, DH, 7, W) int8
    (xor 0x80 fused into the strided transpose copy)."""
    v = u8buf.reshape(2, 128, D, 8, 7, W).transpose(3, 1, 0, 2, 4, 5)
    np.bitwise_xor(v[:, :, :, DH * dh:DH * dh + DH], 0x80,
                   out=out_i8.view(np.uint8).reshape(
                       N_CORES, 128, 2, DH, 7, W))
    return out_i8


def _build_launcher(nc):
    """Persistent jitted SPMD launcher for the compiled program
    (mirrors bass2jax.run_bass_via_pjrt, but the jit is built once).
    dispatch() is async; fetch_outs() blocks and recycles the output
    buffers for future donation."""
    import jax
    from jax.experimental.shard_map import shard_map
    from jax.sharding import Mesh, PartitionSpec
    from concourse.bass2jax import (_bass_exec_p, install_neuronx_cc_hook,
                                    partition_id_tensor)

    install_neuronx_cc_hook()
    partition_name = (nc.partition_id_tensor.name
                      if nc.partition_id_tensor else None)
    in_names, out_names, out_avals, zero_outs = [], [], [], []
    for alloc in nc.m.functions[0].allocations:
        if not isinstance(alloc, mybir.MemoryLocationSet):
            continue
        name = alloc.memorylocations[0].name
        if alloc.kind == "ExternalInput":
            if name != partition_name:
                in_names.append(name)
        elif alloc.kind == "ExternalOutput":
            out_names.append(name)
            shape = tuple(alloc.tensor_shape)
            dtype = mybir.dt.np(alloc.dtype)
            out_avals.append(jax.core.ShapedArray(shape, dtype))
            zero_outs.append(np.zeros((N_CORES * shape[0], *shape[1:]), dtype))
    n_params = len(in_names)
    all_names = list(in_names) + list(out_names)
    if partition_name is not None:
        all_names.append(partition_name)
    donate = tuple(range(n_params, n_params + len(out_names)))

    def _body(*args):
        operands = list(args)
        if partition_name is not None:
            operands.append(partition_id_tensor())
        return tuple(_bass_exec_p.bind(
            *operands,
            out_avals=tuple(out_avals),
            in_names=tuple(all_names),
            out_names=tuple(out_names),
            lowering_input_output_aliases=(),
            sim_require_finite=True,
            sim_require_nnan=True,
            nc=nc,
        ))

    devices = jax.devices()[:N_CORES]
    mesh = Mesh(np.asarray(devices), ("core",))
    sharding = jax.sharding.NamedSharding(mesh, PartitionSpec("core"))
    nin = n_params + len(out_names)
    sharded = jax.jit(
        shard_map(_body, mesh=mesh,
                  in_specs=(PartitionSpec("core"),) * nin,
                  out_specs=(PartitionSpec("core"),) * len(out_names),
                  check_rep=False),
        donate_argnums=donate, keep_unused=True)

    state = {"spare": deque()}

    def put(arr):
        return jax.device_put(arr, sharding)  # async

    def dispatch(concat_inputs):
        """Async launch; returns device output arrays."""
        args = [concat_inputs[n] for n in in_names]
        if state["spare"]:
            dargs = state["spare"].popleft()
        else:
            dargs = [np.zeros_like(z) for z in zero_outs]
        return sharded(*args, *dargs)

    def fetch_outs(outs):
        """Blocking per-shard D2H; recycles outs for future donation."""
        import concurrent.futures as cf

        host = []
        for arr in outs:
            out = np.empty(arr.shape, arr.dtype)
            shards = arr.addressable_shards

            def one(s):
                out[s.index] = np.asarray(s.data)

            with cf.ThreadPoolExecutor(max_workers=len(shards)) as ex:
                list(ex.map(one, shards))
            host.append(out)
        state["spare"].append(list(outs))
        return dict(zip(out_names, host))

    return dispatch, fetch_outs, put


def _get_state():
    if "nc" not in _STATE:
        t0 = time.time()
        _STATE["nc"] = build_mix_program()
        _STATE["build_s"] = time.time() - t0
    return _STATE


def _dummy_inputs():
    dummy_in = {
        "input": np.zeros((B, C, D, H, W), np.float32),
        "n1w": np.ones(C, np.float32), "n1b": np.zeros(C, np.float32),
        "n2w": np.ones(C, np.float32), "n2b": np.zeros(C, np.float32),
        "wqkv": np.zeros((3 * C, C), np.float32),
        "wprojw": np.zeros((C, C), np.float32),
        "wprojb": np.zeros(C, np.float32),
        "wbias": np.zeros((507, 4), np.float32),
        "gqkv": np.zeros((3 * C, C), np.float32),
        "gprojw": np.zeros((C, C), np.float32),
        "gprojb": np.zeros(C, np.float32),
        "gbias": np.zeros((507, 4), np.float32),
    }
    for wk, bk in (("f1c1w", "f1c1b"), ("f1c2w", "f1c2b"),
                   ("g1c1w", "g1c1b"), ("g1c2w", "g1c2b")):
        dummy_in[wk] = np.zeros((128, 128, 3, 3, 3), np.float32)
        dummy_in[bk] = np.zeros(128, np.float32)
    return dummy_in


def _pack_and_run(inputs, put, dispatch):
    """Pack + device_put + dispatch both batches in wire order (blob
    first, then per-batch input chunks + svec as they are quantized).
    Returns the two in-flight output tuples."""
    prof = os.environ.get("MIXBLOCK_PROF")
    t0 = time.monotonic()
    inp = np.asarray(inputs["input"], dtype=np.float32)
    wsh = _pack_blob(inputs)
    dwsh = put(wsh)
    if prof:
        print(f"[prof] blob packed+put at {1e3*(time.monotonic()-t0):.0f} ms")
    tmp = np.empty((C, D, H, W), np.float32)
    u8buf = np.empty((C, D, H, W), np.uint8)
    pending = []
    for b in range(B):
        rsb = _quant_batch(inp[b], tmp, u8buf)
        concat = {"wshb": dwsh}
        for dh in range(2):
            chunk = np.empty((N_CORES * 128, 2, DH, 7, W), np.int8)
            _scatter_chunk(u8buf, dh, chunk)
            concat[f"xq{dh}"] = put(chunk)
        concat["svec"] = put(_make_svec(inputs, rsb))
        pending.append(dispatch(concat))
        if prof:
            print(f"[prof] dispatch{b} issued at "
                  f"{1e3*(time.monotonic()-t0):.0f} ms")
    return pending


def _warmup():
    st = _get_state()
    if "dispatch" in st or os.environ.get("MIXBLOCK_BACKEND") == "sim":
        return
    t0 = time.time()
    st["dispatch"], st["fetch"], st["put"] = _build_launcher(st["nc"])
    dummy_in = _dummy_inputs()
    for _ in range(2):
        pending = _pack_and_run(dummy_in, st["put"], st["dispatch"])
        for outs in pending:
            st["fetch"](outs)
    st["warm_s"] = time.time() - t0


def _run_sim(concat_inputs):
    from concourse.bass_interp import MultiCoreSim
    st = _get_state()
    sim = MultiCoreSim(st["nc"], num_cores=N_CORES,
                       num_workers=int(os.environ.get("MIXBLOCK_SIM_WORKERS",
                                                      "8")))
    for c in range(N_CORES):
        for n, arr in concat_inputs.items():
            per = arr.shape[0] // N_CORES
            sim.cores[c].tensor(n)[:] = arr[c * per:(c + 1) * per]
    sim.simulate()
    outs = np.stack([np.array(sim.cores[c].tensor("outq"))
                     for c in range(N_CORES)])
    return {"outq": outs.reshape(N_CORES * 128, OQC)}


def _unpack(out, b, oq, inp_unused=None):
    """oq (N_CORES*128, OQC) int8 for batch b -> dequant into out[b]."""
    osc = np.ascontiguousarray(oq[:, ODC:OQC]).view(np.float32)  # (1024, 1)
    for c in range(N_CORES):
        src = oq[c * 128:(c + 1) * 128, 0:ODC].reshape(128, 2, D, HQ, W)
        view = out[b].reshape(2, 128, D, H, W)[
            :, :, :, HQ * c:HQ * c + HQ, :].transpose(1, 0, 2, 3, 4)
        np.multiply(src, osc[c * 128:(c + 1) * 128, :, None, None, None],
                    out=view)


def kernel(**inputs):
    LAST_EXEC_NS.clear()
    inp = np.asarray(inputs["input"], dtype=np.float32)
    sim_mode = os.environ.get("MIXBLOCK_BACKEND") == "sim"
    out = np.empty((B, C, D, H, W), np.float32)
    if sim_mode:
        wsh = _pack_blob(inputs)
        tmp = np.empty((C, D, H, W), np.float32)
        u8buf = np.empty((C, D, H, W), np.uint8)
        for b in range(B):
            rsb = _quant_batch(inp[b], tmp, u8buf)
            concat = {"wshb": wsh, "svec": _make_svec(inputs, rsb)}
            for dh in range(2):
                chunk = np.empty((N_CORES * 128, 2, DH, 7, W), np.int8)
                _scatter_chunk(u8buf, dh, chunk)
                concat[f"xq{dh}"] = chunk
            outs = _run_sim(concat)
            _unpack(out, b, outs["outq"].reshape(N_CORES * 128, OQC))
    else:
        _warmup()
        prof = os.environ.get("MIXBLOCK_PROF")
        t0 = time.monotonic()
        pending = _pack_and_run(inputs, _STATE["put"], _STATE["dispatch"])
        hosts = []
        for i, outs in enumerate(pending):
            if prof:
                import jax
                jax.block_until_ready(list(outs))
                print(f"[prof] exec{i} ready at "
                      f"{1e3*(time.monotonic()-t0):.0f} ms")
            hosts.append(_STATE["fetch"](outs))
            if prof:
                print(f"[prof] fetch{i} done at "
                      f"{1e3*(time.monotonic()-t0):.0f} ms")
        LAST_EXEC_NS.append(int((time.monotonic() - t0) * 1e9))
        for b in range(B):
            _unpack(out, b, hosts[b]["outq"].reshape(N_CORES * 128, OQC))
    np.add(out, inp, out=out)
    return out


if os.environ.get("MIXBLOCK_NO_WARMUP") != "1":
    try:
        _warmup()
    except Exception as _e:  # pragma: no cover - fall back to lazy init
        sys.stderr.write(f"mixblock warmup deferred: {_e}\n")
